# revision 31
# baseline (speedup 1.0000x reference)
"""Trainium2 Bass kernel for JonbertaSelfAttention (B=4,S=1024,DM=1024,H=16,D=64,SE=512,DF=512).

Sharding: 8 cores = (batch b = c//2) x (query-half qh = c%2), ONE NEFF for all
cores: the query-half offset l0 is folded into per-core DATA (distance tables
shifted by l0 on the host; the query column slice of hidden_states sent as its
own input) so the same program runs on all 8 cores in a single SPMD launch.

Layout strategy: transposed scores S^T[r_part, l_free]; softmax sums via a
ones-column appended to V in the PV matmul; relative-position bias terms
computed as banded matmuls against the (flipped, per-core-shifted) distance
embedding table and diagonal-extracted via a DRAM round-trip with
per-partition-skewed access patterns; the query-side bias is gathered l-major
and folded into the score accumulation with PE transposes.
"""
import os
import numpy as np
import ml_dtypes

BF16 = ml_dtypes.bfloat16
F8 = ml_dtypes.float8_e4m3
B, S, DM, H, D, SE, DF, MAXP = 4, 1024, 1024, 16, 64, 512, 512, 1024
L = 512          # query rows per core
NRT = S // 128   # 8 r-tiles
NLT = L // 128   # 4 l-tiles
NET = SE // 128  # 4 encoder r-tiles
LN_EPS = 1e-12

_CACHE = {}
LAST_EXEC_NS = None
LAST_MEAN_EXEC_NS = None
LAST_TRACE = None


def _build():
    import concourse.bass as bass
    import concourse.mybir as mybir
    import concourse.tile as tile
    from concourse import bacc
    from concourse.masks import make_identity
    from contextlib import ExitStack

    dt = mybir.dt
    nc = bacc.Bacc("TRN2", target_bir_lowering=False, debug=False, num_devices=8)

    d_hsT = nc.dram_tensor("hsT", [DM, S], dt.float8e4, kind="ExternalInput")
    d_hsTq = nc.dram_tensor("hsTq", [DM, L], dt.float8e4, kind="ExternalInput")
    d_hsres = nc.dram_tensor("hsres", [L, DM], dt.float32, kind="ExternalInput")
    d_encT = nc.dram_tensor("encT", [DF, SE], dt.float8e4, kind="ExternalInput")
    d_mask = nc.dram_tensor("mask", [S], dt.float32, kind="ExternalInput")
    d_wqT = nc.dram_tensor("wqT", [DM, DM], dt.float8e4, kind="ExternalInput")
    d_wkT = nc.dram_tensor("wkT", [DM, DM], dt.float8e4, kind="ExternalInput")
    d_wvT = nc.dram_tensor("wvT", [DM, DM], dt.float8e4, kind="ExternalInput")
    d_wfkT = nc.dram_tensor("wfkT", [DF, DM], dt.float8e4, kind="ExternalInput")
    d_wfvT = nc.dram_tensor("wfvT", [DF, DM], dt.float8e4, kind="ExternalInput")
    d_woT = nc.dram_tensor("woT", [DM, DM], dt.float8e4, kind="ExternalInput")
    d_bq = nc.dram_tensor("bq", [DM], dt.float32, kind="ExternalInput")
    d_bk = nc.dram_tensor("bk", [DM], dt.float32, kind="ExternalInput")
    d_bfk = nc.dram_tensor("bfk", [DM], dt.float32, kind="ExternalInput")
    d_bv = nc.dram_tensor("bv", [DM], dt.bfloat16, kind="ExternalInput")
    d_bfv = nc.dram_tensor("bfv", [DM], dt.bfloat16, kind="ExternalInput")
    d_bo = nc.dram_tensor("bo", [DM], dt.float32, kind="ExternalInput")
    d_lng = nc.dram_tensor("lng", [DM], dt.float32, kind="ExternalInput")
    d_lnb = nc.dram_tensor("lnb", [DM], dt.float32, kind="ExternalInput")
    d_distn = nc.dram_tensor("distn", [D, 2048], dt.bfloat16, kind="ExternalInput")
    d_distf = nc.dram_tensor("distf", [D, 2048], dt.bfloat16, kind="ExternalInput")
    d_out = nc.dram_tensor("out", [L, DM], dt.float32, kind="ExternalOutput")

    AP = bass.AP
    f32 = dt.float32
    bf16 = dt.bfloat16
    fp8g = dt.float8e4
    AF = mybir.ActivationFunctionType

    with tile.TileContext(nc) as tc, ExitStack() as top:
        scr = top.enter_context(tc.tile_pool(name="scr", bufs=H, space="DRAM"))
        scr2 = top.enter_context(tc.tile_pool(name="scr2", bufs=H, space="DRAM"))
        scr3 = top.enter_context(tc.tile_pool(name="scr3", bufs=6, space="DRAM"))
        pers = top.enter_context(tc.tile_pool(name="pers", bufs=1))
        kT = pers.tile([128, NRT, S], bf16, tag="kT")
        qT = pers.tile([128, NRT, L], bf16, tag="qT")
        fkT = pers.tile([128, NRT, SE], bf16, tag="fkT")
        v_sb = pers.tile([128, NRT, H, 65], bf16, tag="v_sb")
        fv_sb = pers.tile([128, NET, H, 65], bf16, tag="fv_sb")
        hsres = pers.tile([128, NLT, DM], f32, tag="hsres")
        ctxpk = pers.tile([64, NRT, 2, L], bf16, tag="ctxpk")
        bv_b = pers.tile([128, DM], bf16, tag="bv_b")
        bfv_b = pers.tile([128, DM], bf16, tag="bfv_b")
        lng_b = pers.tile([128, DM], f32, tag="lng_b")
        lnb_b = pers.tile([128, DM], f32, tag="lnb_b")
        bo_b = pers.tile([128, DM], f32, tag="bo_b")
        bq_s = pers.tile([128, NRT], f32, tag="bq_s")
        bk_s = pers.tile([128, NRT], f32, tag="bk_s")
        bfk_s = pers.tile([128, NRT], f32, tag="bfk_s")
        msk = pers.tile([128, NRT], f32, tag="msk")
        ident = pers.tile([128, 128], bf16, tag="ident")
        eps_t = pers.tile([128, 1], f32, tag="eps_t")
        zero_t = pers.tile([128, 1], f32, tag="zero_t")

        make_identity(nc, ident[:])
        nc.vector.memset(eps_t[:], LN_EPS)
        nc.vector.memset(zero_t[:], 0.0)
        nc.sync.dma_start(out=bq_s[:], in_=AP(tensor=d_bq, offset=0, ap=[[1, 128], [128, NRT]]))
        nc.sync.dma_start(out=bk_s[:], in_=AP(tensor=d_bk, offset=0, ap=[[1, 128], [128, NRT]]))
        nc.sync.dma_start(out=bfk_s[:], in_=AP(tensor=d_bfk, offset=0, ap=[[1, 128], [128, NRT]]))
        nc.sync.dma_start(out=msk[:], in_=AP(tensor=d_mask, offset=0, ap=[[1, 128], [128, NRT]]))
        nc.vector.memset(v_sb[:, :, :, 64:65], 1.0)
        nc.vector.memset(fv_sb[:, :, :, 64:65], 1.0)

        cq_dram = {}
        ck_dram = {}

        fp8 = dt.float8e4
        with ExitStack() as phB:
            pb = phB.enter_context(tc.tile_pool(name="pb", bufs=1))
            hsT = pb.tile([128, NRT, S], fp8g, tag="hsT")
            hsTq = pb.tile([128, NRT, L], fp8g, tag="hsTq")
            encT = pb.tile([128, NET, SE], fp8g, tag="encT")
            wv_k = pb.tile([128, NRT, DM], fp8g, tag="wv_k")
            wfv_k = pb.tile([128, NET, DM], fp8g, tag="wfv_k")
            distn_s = pb.tile([128, 2048], bf16, tag="distn")
            distf_s = pb.tile([128, 2048], bf16, tag="distf")
            wst = phB.enter_context(tc.tile_pool(name="wst", bufs=3))
            bsb = phB.enter_context(tc.tile_pool(name="bsb", bufs=2))
            bsb2 = phB.enter_context(tc.tile_pool(name="bsb2", bufs=2))
            pp_proj = phB.enter_context(tc.tile_pool(name="pp_proj", bufs=2, space="PSUM"))
            pp_band = phB.enter_context(tc.tile_pool(name="pp_band", bufs=2, space="PSUM"))

            # order: what the first projections need comes first; V/FV weights
            # (used only after the ot loop) load behind the band traffic
            nc.sync.dma_start(out=hsTq[:], in_=AP(tensor=d_hsTq, offset=0,
                                                  ap=[[L, 128], [128 * L, NRT], [1, L]]))
            nc.sync.dma_start(out=hsT[:], in_=AP(tensor=d_hsT, offset=0,
                                                 ap=[[S, 128], [128 * S, NRT], [1, S]]))
            nc.sync.dma_start(out=encT[:], in_=AP(tensor=d_encT, offset=0,
                                                  ap=[[SE, 128], [128 * SE, NET], [1, SE]]))
            for half in range(2):
                nc.sync.dma_start(out=distn_s[half * 64:(half + 1) * 64, :],
                                  in_=AP(tensor=d_distn, offset=0, ap=[[2048, 64], [1, 2048]]))
                nc.sync.dma_start(out=distf_s[half * 64:(half + 1) * 64, :],
                                  in_=AP(tensor=d_distf, offset=0, ap=[[2048, 64], [1, 2048]]))

            def emit_bands(h):
                hp = (h % 2) * 64
                ot = h // 2
                cq = scr.tile([NLT * 128, 1152], fp8, tag="cq")
                cq_dram[h] = cq
                qstage = bsb.tile([128, NLT, 1152], fp8, tag="qstage")
                for lt in range(NLT):
                    bm = 896 - lt * 128
                    ps = pp_band.tile([128, 1152], f32, tag="band")
                    for n0, nn in ((0, 512), (512, 512), (1024, 128)):
                        nc.tensor.matmul(ps[:, n0:n0 + nn],
                                         lhsT=qT[hp:hp + 64, ot, lt * 128:(lt + 1) * 128],
                                         rhs=distf_s[hp:hp + 64, bm + n0:bm + n0 + nn],
                                         start=True, stop=True)
                    nc.scalar.copy(out=qstage[:, lt, :], in_=ps[:])
                nc.sync.dma_start(out=AP(tensor=cq.tensor, offset=cq.offset,
                                         ap=[[1152, 128], [128 * 1152, NLT], [1, 1152]]),
                                  in_=qstage[:])
                ck = scr2.tile([NRT * 128, 640], fp8, tag="ck")
                ck_dram[h] = ck
                kstage = bsb2.tile([128, NRT, 640], fp8, tag="kstage")
                for rt in range(NRT):
                    bt = 896 - 128 * rt
                    ps = pp_band.tile([128, 1152], f32, tag="band")
                    for n0, nn in ((0, 512), (512, 128)):
                        nc.tensor.matmul(ps[:, n0:n0 + nn],
                                         lhsT=kT[hp:hp + 64, ot, rt * 128:(rt + 1) * 128],
                                         rhs=distn_s[hp:hp + 64, bt + n0:bt + n0 + nn],
                                         start=True, stop=True)
                    if rt < 6:
                        nc.vector.scalar_tensor_tensor(
                            out=kstage[:, rt, :], in0=ps[:, 0:640], scalar=msk[:, rt:rt + 1],
                            in1=hsT[:, 0, 0:640], op0=mybir.AluOpType.add,
                            op1=mybir.AluOpType.bypass)
                    else:
                        nc.scalar.activation(out=kstage[:, rt, :], in_=ps[:, 0:640],
                                             func=AF.Identity, bias=msk[:, rt:rt + 1],
                                             scale=1.0)
                nc.sync.dma_start(out=AP(tensor=ck.tensor, offset=ck.offset,
                                         ap=[[640, 128], [128 * 640, NRT], [1, 640]]),
                                  in_=kstage[:])

            for ot in range(NRT):
                # qT o-tile (local half of queries, via pre-sliced hsTq)
                wq_ot = wst.tile([128, NRT, 128], fp8g, tag="wblk")
                nc.sync.dma_start(out=wq_ot[:], in_=AP(
                    tensor=d_wqT, offset=ot * 128,
                    ap=[[DM, 128], [128 * DM, NRT], [1, 128]]))
                ps = pp_proj.tile([128, 512], f32, tag="proj")
                for it in range(NRT):
                    nc.tensor.matmul(ps[:], lhsT=wq_ot[:, it, :], rhs=hsTq[:, it, :],
                                     start=(it == 0), stop=(it == NRT - 1))
                nc.scalar.activation(out=qT[:, ot, :], in_=ps[:], func=AF.Identity,
                                     bias=bq_s[:, ot:ot + 1], scale=1.0)
                # kT o-tile (full sequence)
                wk_ot = wst.tile([128, NRT, 128], fp8g, tag="wblk")
                nc.sync.dma_start(out=wk_ot[:], in_=AP(
                    tensor=d_wkT, offset=ot * 128,
                    ap=[[DM, 128], [128 * DM, NRT], [1, 128]]))
                for sb_i in range(2):
                    ps = pp_proj.tile([128, 512], f32, tag="proj")
                    for it in range(NRT):
                        nc.tensor.matmul(ps[:], lhsT=wk_ot[:, it, :], rhs=hsT[:, it, sb_i * 512:(sb_i + 1) * 512],
                                         start=(it == 0), stop=(it == NRT - 1))
                    nc.scalar.activation(out=kT[:, ot, sb_i * 512:(sb_i + 1) * 512], in_=ps[:],
                                         func=AF.Identity, bias=bk_s[:, ot:ot + 1], scale=1.0)
                # fkT o-tile
                wfk_ot = wst.tile([128, NET, 128], fp8g, tag="wblk2")
                nc.sync.dma_start(out=wfk_ot[:], in_=AP(
                    tensor=d_wfkT, offset=ot * 128,
                    ap=[[DM, 128], [128 * DM, NET], [1, 128]]))
                ps = pp_proj.tile([128, 512], f32, tag="proj")
                for it in range(NET):
                    nc.tensor.matmul(ps[:], lhsT=wfk_ot[:, it, :], rhs=encT[:, it, :],
                                     start=(it == 0), stop=(it == NET - 1))
                nc.scalar.activation(out=fkT[:, ot, :], in_=ps[:], func=AF.Identity,
                                     bias=bfk_s[:, ot:ot + 1], scale=1.0)
                emit_bands(2 * ot)
                emit_bands(2 * ot + 1)

            nc.sync.dma_start(out=bv_b[:], in_=AP(tensor=d_bv, offset=0, ap=[[0, 128], [1, DM]]))
            nc.sync.dma_start(out=bfv_b[:], in_=AP(tensor=d_bfv, offset=0, ap=[[0, 128], [1, DM]]))
            nc.sync.dma_start(out=wv_k[:], in_=AP(tensor=d_wvT, offset=0,
                                                  ap=[[DM, 128], [128 * DM, NRT], [1, DM]]))
            nc.sync.dma_start(out=wfv_k[:], in_=AP(tensor=d_wfvT, offset=0,
                                                   ap=[[DM, 128], [128 * DM, NET], [1, DM]]))
            # V projection (s-major) and FV
            for st in range(NRT):
                for ob in range(2):
                    ps = pp_proj.tile([128, 512], f32, tag="proj")
                    for it in range(NRT):
                        nc.tensor.matmul(ps[:], lhsT=hsT[:, it, st * 128:(st + 1) * 128],
                                         rhs=wv_k[:, it, ob * 512:(ob + 1) * 512],
                                         start=(it == 0), stop=(it == NRT - 1))
                    nc.vector.tensor_add(
                        out=v_sb[:, st, ob * 8:(ob + 1) * 8, 0:64],
                        in0=ps[:].rearrange("p (h d) -> p h d", d=64),
                        in1=bv_b[:, ob * 512:(ob + 1) * 512].rearrange("p (h d) -> p h d", d=64))
            for st in range(NET):
                for ob in range(2):
                    ps = pp_proj.tile([128, 512], f32, tag="proj")
                    for it in range(NET):
                        nc.tensor.matmul(ps[:], lhsT=encT[:, it, st * 128:(st + 1) * 128],
                                         rhs=wfv_k[:, it, ob * 512:(ob + 1) * 512],
                                         start=(it == 0), stop=(it == NET - 1))
                    nc.vector.tensor_add(
                        out=fv_sb[:, st, ob * 8:(ob + 1) * 8, 0:64],
                        in0=ps[:].rearrange("p (h d) -> p h d", d=64),
                        in1=bfv_b[:, ob * 512:(ob + 1) * 512].rearrange("p (h d) -> p h d", d=64))

        # ---------- attention phase ----------
        with ExitStack() as phC:
            gp = phC.enter_context(tc.tile_pool(name="gp", bufs=2))
            g2 = phC.enter_context(tc.tile_pool(name="g2", bufs=2))
            ep = phC.enter_context(tc.tile_pool(name="ep", bufs=4))
            cp = phC.enter_context(tc.tile_pool(name="cp", bufs=4))
            rp = phC.enter_context(tc.tile_pool(name="rp", bufs=4))
            pp_s = phC.enter_context(tc.tile_pool(name="pp_s", bufs=2, space="PSUM"))
            pp_c = phC.enter_context(tc.tile_pool(name="pp_c", bufs=4, space="PSUM"))

            for h in range(H):
                hp = (h % 2) * 64
                ot = h // 2
                b1all = gp.tile([128, NLT, 1024], fp8, tag="b1all")
                src = cq_dram[h]
                nc.sync.dma_start(out=b1all[:], in_=AP(
                    tensor=src.tensor, offset=src.offset + 127,
                    ap=[[1151, 128], [128 * 1152, NLT], [1, 1024]]))
                b2all = g2.tile([128, NRT, 512], fp8, tag="b2all")
                src = ck_dram[h]
                nc.sync.dma_start(out=b2all[:], in_=AP(
                    tensor=src.tensor, offset=src.offset + 127,
                    ap=[[639, 128], [128 * 640, NRT], [1, 512]]))
                ctx_ps = pp_c.tile([65, 512], f32, tag="ctx")
                ctxe_ps = pp_c.tile([65, 512], f32, tag="ctx")
                def scores_pair(u):
                    # mask is pre-folded into b2all (k-band copy); exp has zero bias
                    ps = pp_s.tile([128, 1024], f32, tag="sc")
                    for half in range(2):
                        rt = 2 * u + half
                        o0 = half * 512
                        nc.tensor.matmul(ps[:, o0:o0 + 512],
                                         lhsT=kT[hp:hp + 64, ot, rt * 128:(rt + 1) * 128],
                                         rhs=qT[hp:hp + 64, ot, :], start=True, stop=False,
                                         skip_group_check=True)
                        for lt in range(NLT):
                            nc.tensor.matmul(ps[:, o0 + lt * 128:o0 + (lt + 1) * 128],
                                             lhsT=b1all[:, lt, rt * 128:(rt + 1) * 128],
                                             rhs=ident[:], start=False, stop=False,
                                             skip_group_check=True)
                        nc.tensor.matmul(ps[:, o0:o0 + 512], lhsT=ident[:],
                                         rhs=b2all[:, rt, :],
                                         start=False, stop=True, skip_group_check=True)
                    return ps

                def exp_pair(ps):
                    ex = ep.tile([128, 1024], bf16, tag="ex")
                    nc.scalar.activation(out=ex[:], in_=ps[:], func=AF.Exp,
                                         bias=zero_t[:], scale=0.125)
                    return ex

                def pv_pair(ex, u, last):
                    for half in range(2):
                        rt = 2 * u + half
                        nc.tensor.matmul(ctx_ps[:], lhsT=v_sb[:, rt, h, :],
                                         rhs=ex[:, half * 512:(half + 1) * 512],
                                         start=(rt == 0), stop=(last and half == 1),
                                         skip_group_check=True)

                # PV runs one pair behind the scores, hiding exp latency
                exs = {}
                for u in range(4):
                    ps = scores_pair(u)
                    if u > 0:
                        pv_pair(exs[u - 1], u - 1, False)
                    exs[u] = exp_pair(ps)

                def enc_scores_pair(p):
                    ps = pp_s.tile([128, 1024], f32, tag="sc")
                    for half in range(2):
                        ret = 2 * p + half
                        nc.tensor.matmul(
                            ps[:, half * 512:(half + 1) * 512],
                            lhsT=fkT[hp:hp + 64, ot, ret * 128:(ret + 1) * 128],
                            rhs=qT[hp:hp + 64, ot, :], start=True, stop=True,
                            skip_group_check=True)
                    return ps

                eps0 = enc_scores_pair(0)
                pv_pair(exs[3], 3, True)
                eex0 = exp_pair(eps0)
                eps1 = enc_scores_pair(1)
                for half in range(2):
                    nc.tensor.matmul(ctxe_ps[:], lhsT=fv_sb[:, half, h, :],
                                     rhs=eex0[:, half * 512:(half + 1) * 512],
                                     start=(half == 0), stop=False, skip_group_check=True)
                eex1 = exp_pair(eps1)
                for half in range(2):
                    nc.tensor.matmul(ctxe_ps[:], lhsT=fv_sb[:, 2 + half, h, :],
                                     rhs=eex1[:, half * 512:(half + 1) * 512],
                                     start=False, stop=(half == 1), skip_group_check=True)
                # normalize + combine: bounce reciprocal rows via DRAM for broadcast
                dr = scr3.tile([2, 512], f32, tag="recd")
                rec2a = rp.tile([1, 512], f32, tag="rec2a")
                rec2b = rp.tile([1, 512], f32, tag="rec2b")
                nc.vector.reciprocal(out=rec2a[:], in_=ctx_ps[64:65, :])
                nc.vector.reciprocal(out=rec2b[:], in_=ctxe_ps[64:65, :])
                nc.sync.dma_start(out=dr[0:1, :], in_=rec2a[:])
                nc.sync.dma_start(out=dr[1:2, :], in_=rec2b[:])
                rb = rp.tile([64, 2, 512], f32, tag="rb")
                nc.sync.dma_start(out=rb[:], in_=AP(tensor=dr.tensor, offset=dr.offset,
                                                    ap=[[0, 64], [512, 2], [1, 512]]))
                rb1 = rb[:, 0, :]
                rb2 = rb[:, 1, :]
                t1 = cp.tile([64, 512], f32, tag="t1")
                t2 = cp.tile([64, 512], f32, tag="t2")
                nc.vector.tensor_mul(out=t1[:], in0=ctx_ps[0:64, :], in1=rb1)
                nc.vector.tensor_mul(out=t2[:], in0=ctxe_ps[0:64, :], in1=rb2)
                nc.vector.tensor_add(out=ctxpk[:, ot, h % 2, :], in0=t1[:], in1=t2[:])

        # ---------- output dense + residual + LN ----------
        # phase-D-only inputs load here: off the startup critical path
        nc.sync.dma_start(out=lng_b[:], in_=AP(tensor=d_lng, offset=0, ap=[[0, 128], [1, DM]]))
        nc.sync.dma_start(out=lnb_b[:], in_=AP(tensor=d_lnb, offset=0, ap=[[0, 128], [1, DM]]))
        nc.sync.dma_start(out=bo_b[:], in_=AP(tensor=d_bo, offset=0, ap=[[0, 128], [1, DM]]))
        nc.sync.dma_start(out=hsres[:], in_=AP(tensor=d_hsres, offset=0,
                                               ap=[[DM, 128], [128 * DM, NLT], [1, DM]]))
        for st in range(NLT):
            nc.vector.tensor_add(out=hsres[:, st, :], in0=hsres[:, st, :], in1=bo_b[:])

        with ExitStack() as phD:
            pd = phD.enter_context(tc.tile_pool(name="pd", bufs=1))
            wo_sb = pd.tile([64, H, DM], fp8g, tag="wo_sb")
            yp = phD.enter_context(tc.tile_pool(name="yp", bufs=2))
            op = phD.enter_context(tc.tile_pool(name="op", bufs=2))
            stp = phD.enter_context(tc.tile_pool(name="stp", bufs=2))
            pp_y = phD.enter_context(tc.tile_pool(name="pp_y", bufs=2, space="PSUM"))

            nc.sync.dma_start(out=wo_sb[:], in_=AP(tensor=d_woT, offset=0,
                                                   ap=[[DM, 64], [64 * DM, H], [1, DM]]))
            for st in range(NLT):
                y = yp.tile([128, DM], f32, tag="y")
                for ob in range(2):
                    ps = pp_y.tile([128, 512], f32, tag="py")
                    for hh in range(H):
                        nc.tensor.matmul(
                            ps[:], lhsT=ctxpk[:, hh // 2, hh % 2, st * 128:(st + 1) * 128],
                            rhs=wo_sb[:, hh, ob * 512:(ob + 1) * 512],
                            start=(hh == 0), stop=(hh == H - 1))
                    nc.vector.tensor_add(out=y[:, ob * 512:(ob + 1) * 512], in0=ps[:],
                                         in1=hsres[:, st, ob * 512:(ob + 1) * 512])
                stats = stp.tile([128, 2, 6], f32, tag="stats")
                nc.vector.bn_stats(out=stats[:, 0, :], in_=y[:, 0:512])
                nc.vector.bn_stats(out=stats[:, 1, :], in_=y[:, 512:1024])
                mv = stp.tile([128, 2], f32, tag="mv")
                nc.vector.bn_aggr(out=mv[:], in_=stats[:])
                sd = stp.tile([128, 1], f32, tag="sd")
                nc.scalar.activation(out=sd[:], in_=mv[:, 1:2], func=AF.Sqrt,
                                     bias=eps_t[:], scale=1.0)
                rstd = stp.tile([128, 1], f32, tag="rstd")
                nc.vector.reciprocal(out=rstd[:], in_=sd[:])
                o1 = op.tile([128, DM], f32, tag="o1")
                nc.vector.tensor_scalar(out=o1[:], in0=y[:], scalar1=mv[:, 0:1], scalar2=rstd[:],
                                        op0=mybir.AluOpType.subtract, op1=mybir.AluOpType.mult)
                o2 = op.tile([128, DM], f32, tag="o2")
                nc.vector.tensor_mul(out=o2[:], in0=o1[:], in1=lng_b[:])
                o3 = op.tile([128, DM], f32, tag="o3")
                nc.vector.tensor_add(out=o3[:], in0=o2[:], in1=lnb_b[:])
                nc.sync.dma_start(out=d_out[st * 128:(st + 1) * 128, :], in_=o3[:])

    nc.finalize()
    return nc


def _get_nc():
    if "nc" not in _CACHE:
        _CACHE["nc"] = _build()
    return _CACHE["nc"]


def kernel(**inputs):
    global LAST_EXEC_NS, LAST_MEAN_EXEC_NS, LAST_TRACE
    from concourse.bass_utils import run_bass_kernel_spmd

    inp = {k: np.asarray(v) for k, v in inputs.items()}
    hs = inp["hidden_states"].astype(np.float32)
    mask = inp["attention_mask"].astype(np.float32)
    enc = inp["encoder_hidden_states"].astype(np.float32)
    G = inp["dist_emb"].astype(np.float32)

    def b16(x):
        return np.ascontiguousarray(x.astype(BF16))

    def f8(x):
        return np.ascontiguousarray(x.astype(F8))

    shared = {
        "wqT": f8(inp["Wq"].T), "wkT": f8(inp["Wk"].T), "wvT": f8(inp["Wv"].T),
        "wfkT": f8(inp["Wfk"].T), "wfvT": f8(inp["Wfv"].T), "woT": f8(inp["Wo"].T),
        "bq": inp["bq"].astype(np.float32), "bk": inp["bk"].astype(np.float32),
        "bfk": inp["bfk"].astype(np.float32), "bv": b16(inp["bv"]), "bfv": b16(inp["bfv"]),
        "bo": inp["bo"].astype(np.float32), "lng": inp["ln_g"].astype(np.float32),
        "lnb": inp["ln_b"].astype(np.float32),
    }
    # Padded tables: G' (natural order) and F' (flipped), plus per-query-half
    # shifted variants so one NEFF (band offsets hardcoded for l0=0) serves
    # both query halves.
    Gp = np.zeros((2048, D), np.float32); Gp[:2047] = G
    Fp = np.zeros((2048, D), np.float32); Fp[:2047] = G[::-1]
    distn_q = {0: Gp, 1: np.zeros((2048, D), np.float32)}
    distf_q = {0: Fp, 1: np.zeros((2048, D), np.float32)}
    distn_q[1][0:1536] = Gp[512:2048]
    distf_q[1][512:2048] = Fp[0:1536]

    in_maps = []
    for c in range(8):
        b, qhc = c // 2, c % 2
        l0 = qhc * L
        m = dict(shared)
        hsTb = hs[b].T
        m["hsT"] = f8(hsTb)
        m["hsTq"] = f8(hsTb[:, l0:l0 + L])
        m["hsres"] = np.ascontiguousarray(hs[b, l0:l0 + L, :])
        m["encT"] = f8(enc[b].T)
        m["mask"] = np.ascontiguousarray(np.broadcast_to(mask[b, 0, 0, :], (S,)) * 8.0)
        m["distn"] = b16(distn_q[qhc].T)
        m["distf"] = b16(distf_q[qhc].T)
        in_maps.append(m)

    nc = _get_nc()
    res = run_bass_kernel_spmd(nc, in_maps, core_ids=list(range(8)))
    LAST_EXEC_NS = res.exec_time_ns
    LAST_MEAN_EXEC_NS = res.mean_exec_time_ns
    LAST_TRACE = res.instructions_and_trace

    out = np.zeros((B, S, DM), np.float32)
    for c in range(8):
        b, qhc = c // 2, c % 2
        out[b, qhc * L:(qhc + 1) * L, :] = res.results[c]["out"]
    return out
